# revision 14
# baseline (speedup 1.0000x reference)
"""DWT enhancement (db4, 9 levels, symmetric padding, universal soft-threshold)
as a Bass/Tile kernel for 8 Trainium2 NeuronCores.

Input x: [128, 4096, 64] f32 (B, S, F); one signal per (batch, feature).
Sharding: 16 batches per core (pure data parallel), processed in 4 blocks of
4 batches = 256 signals (J).  SBUF keeps SAMPLES on partitions and SIGNALS on
the free axis; since F=64 is innermost in HBM, [128 samples, 4*64 signals]
tiles come from clean strided DMAs (256B runs), no transposes anywhere.

Each DWT/IDWT level is a block-banded linear map; 128-row blocks of exact
host-built matrices (symmetric padding + pywt trim folded in) are PE matmuls
contracting over partitions.  std(d1) per signal via ones-matmul column sums;
threshold broadcast across partitions via a rank-1 matmul; soft-threshold
c - clamp(c,-T,T) as tensor ops on DVE (d1, a9) and GPSIMD (d2..d9).

Dtypes: level-1 forward in float32r (fast-fp32 PE mode) on raw f32 x; PSUM
always f32; coefficients evicted as fp16; deeper matmuls fp16; threshold
math f32.
"""

import numpy as np

import concourse.bass as bass
import concourse.bacc as bacc
import concourse.tile as tile
from concourse import mybir
from concourse.bass_utils import run_bass_kernel_spmd

B, S, F = 128, 4096, 64
FLEN = 8
LEVEL = 9
NCORES = 8
BPC = B // NCORES          # 16 batches per core
JB = 4                     # batches per block
J = JB * F                 # 256 signals per block
NBLK = BPC // JB           # 4 blocks per core
CH = 128
GRP = 4                    # psum out-chunks per eviction group

_REC_LO = np.array([0.23037781330885523, 0.7148465705525415, 0.6308807679295904,
                    -0.02798376941698385, -0.18703481171888114, 0.030841381835986965,
                    0.032883011666982945, -0.010597401784997278], dtype=np.float32)
_REC_HI = _REC_LO[::-1].copy(); _REC_HI[1::2] *= -1.0
_DEC_LO = _REC_LO[::-1].copy()
_DEC_HI = _REC_HI[::-1].copy()
THR_K = float(np.sqrt(2.0 * np.log(float(S))))

F32 = mybir.dt.float32
F32R = mybir.dt.float32r
F16 = mybir.dt.float16


def _sym(q, n):
    if q < 0:
        return -q - 1
    if q >= n:
        return 2 * n - 1 - q
    return q


def _lens():
    lens = [S]
    n = S
    for _ in range(LEVEL):
        n = (n + FLEN - 1) // 2
        lens.append(n)
    return lens  # [4096, 2051, 1029, 518, 262, 134, 70, 38, 22, 14]


LENS = _lens()
NCH = [(n + CH - 1) // CH for n in LENS]


def _fwd_mat(n, h):
    m = (n + FLEN - 1) // 2
    M = np.zeros((n, m), dtype=np.float64)
    for k in range(m):
        for j in range(FLEN):
            M[_sym(2 * k + 1 - j, n), k] += h[j]
    return M.astype(np.float32)


def _inv_mat(n, h):
    L = 2 * n - FLEN + 2
    M = np.zeros((n, L), dtype=np.float64)
    for t in range(L):
        for i in range(max(0, (t - 1 + 1) // 2), min(n, t // 2 + 4)):
            f = t + 6 - 2 * i
            if 0 <= f < FLEN:
                M[i, t] = h[f]
    return M.astype(np.float32)


class Plan:
    def __init__(self):
        self.mats = []
        self._index = {}
        self.fwd = []   # fwd[l][part] = list of (mv, [(ic, r0, r1, mid)])
        self.inv = []   # (nd, out_len, [(mv, [(src, ic, r0, r1, mid)])])
        self.rec_lens = []
        self._build()

    def _mat_id(self, m, r0):
        # the matrix is stored in the weight tile at partition offset r0 so
        # that lhsT and rhs share the same base partition
        m = np.ascontiguousarray(m, dtype=np.float32)
        key = (m.shape, r0, m.tobytes())
        mid = self._index.get(key)
        if mid is None:
            mid = len(self.mats)
            self.mats.append((m, r0))
            self._index[key] = mid
        return mid

    def _blockify(self, M):
        n_in, n_out = M.shape
        out = []
        for c in range(0, n_out, CH):
            mv = min(CH, n_out - c)
            entries = []
            for r in range(0, n_in, CH):
                kv = min(CH, n_in - r)
                blk = M[r:r + kv, c:c + mv]
                nz = np.nonzero(np.any(blk != 0.0, axis=1))[0]
                if len(nz) == 0:
                    continue
                r0, r1 = int(nz[0]), int(nz[-1]) + 1
                # matmul operands must start at partition 0, 32, or 64
                r0 = min((r0 // 32) * 32, 64)
                entries.append((r // CH, r0, r1, self._mat_id(blk[r0:r1], r0)))
            out.append((mv, entries))
        return out

    def _build(self):
        for l in range(LEVEL):
            n = LENS[l]
            self.fwd.append({'a': self._blockify(_fwd_mat(n, _DEC_LO)),
                             'd': self._blockify(_fwd_mat(n, _DEC_HI))})
        na = LENS[LEVEL]
        for l in range(LEVEL, 0, -1):
            nd = LENS[l]
            assert na in (nd, nd + 1)
            out_len = 2 * nd - FLEN + 2
            ga = self._blockify(_inv_mat(nd, _REC_LO))
            gd = self._blockify(_inv_mat(nd, _REC_HI))
            merged = []
            for (mv, ea), (_, ed) in zip(ga, gd):
                ents = [('a', ic, r0, r1, mid) for (ic, r0, r1, mid) in ea] + \
                       [('d', ic, r0, r1, mid) for (ic, r0, r1, mid) in ed]
                merged.append((mv, ents))
            self.inv.append((nd, out_len, merged))
            self.rec_lens.append(out_len)
            na = out_len
        assert na == S


PLAN = Plan()


def plan_stats():
    nmm_f = sum(len(e) for lv in PLAN.fwd for p in ('a', 'd') for _, e in lv[p])
    nmm_i = sum(len(e) for _, _, oc in PLAN.inv for _, e in oc)
    wkb16 = sum(m.size for m, _ in PLAN.mats) * 2 / 1024
    return dict(n_mats=len(PLAN.mats), nmm_fwd=nmm_f, nmm_inv=nmm_i,
                weight_kb_fp16=wkb16)


# ---------------------------------------------------------------------------
# Numpy simulation of the exact plan (host-side validation of matrices/flow)
# ---------------------------------------------------------------------------
def simulate_plan(sig, fp16=True):
    cast = (lambda v: v.astype(np.float16).astype(np.float32)) if fp16 else (lambda v: v)
    X = sig.T.astype(np.float32)

    def apply(plan_oc, srcs):
        outs = []
        for mv, entries in plan_oc:
            acc = np.zeros((mv, X.shape[1]), dtype=np.float32)
            for ent in entries:
                if len(ent) == 4:
                    ic, r0, r1, mid = ent
                    src = srcs
                else:
                    s, ic, r0, r1, mid = ent
                    src = srcs[s]
                acc += PLAN.mats[mid][0].T @ src[ic * CH + r0: ic * CH + r1]
            outs.append(acc)
        return np.concatenate(outs, axis=0)

    a = cast(X)
    dets = []
    for l in range(LEVEL):
        srcs = cast(a)
        dets.append(cast(apply(PLAN.fwd[l]['d'], srcs)))
        a = apply(PLAN.fwd[l]['a'], srcs)
    a = cast(a)
    d1 = dets[0]
    n1 = d1.shape[0]
    s1, s2 = d1.sum(axis=0), (d1 * d1).sum(axis=0)
    var = np.maximum(s2 / n1 - (s1 / n1) ** 2, 0.0)
    T = cast(np.sqrt(var) * THR_K)[None, :]

    def soft(c):
        return cast(c - np.minimum(np.maximum(c, -T), T))

    rec = soft(a)
    for i, l in enumerate(range(LEVEL, 0, -1)):
        nd, out_len, oc = PLAN.inv[i]
        rec = apply(oc, {'a': rec[:nd], 'd': soft(dets[l - 1])})
        if i < LEVEL - 1:
            rec = cast(rec)
    return rec.T


# ---------------------------------------------------------------------------
# Bass kernel
# ---------------------------------------------------------------------------
def _groups(ocs):
    """Group out-chunks into runs of GRP full chunks; partial chunks alone."""
    out = []
    cur = []
    for oc, (mv, entries) in enumerate(ocs):
        if mv == CH:
            cur.append(oc)
            if len(cur) == GRP:
                out.append(cur)
                cur = []
        else:
            if cur:
                out.append(cur)
                cur = []
            out.append([oc])
    if cur:
        out.append(cur)
    return out


def _bcast(t, nchk):
    ap = [list(p) for p in t.ap]
    return bass.AP(tensor=t.tensor, offset=t.offset,
                   ap=[ap[0], [0, nchk], ap[1]])


def _kernel_body(tc, y_out, x_in, w16_in, plan):
    nc = tc.nc
    from contextlib import ExitStack
    ctx = ExitStack()
    with ctx:
        consts = ctx.enter_context(tc.tile_pool(name="consts", bufs=1))
        xpool = ctx.enter_context(tc.tile_pool(name="x", bufs=4))
        apool = ctx.enter_context(tc.tile_pool(name="acoef", bufs=1))
        dpool = ctx.enter_context(tc.tile_pool(name="dcoef", bufs=2))
        rpool = ctx.enter_context(tc.tile_pool(name="rec", bufs=1))
        opool = ctx.enter_context(tc.tile_pool(name="outb", bufs=3))
        tpool = ctx.enter_context(tc.tile_pool(name="thr", bufs=2))
        sqpool = ctx.enter_context(tc.tile_pool(name="sq", bufs=2))
        upool = ctx.enter_context(tc.tile_pool(name="clamp", bufs=2))
        psum = ctx.enter_context(tc.tile_pool(name="psum", bufs=3, space="PSUM"))
        psum_s = ctx.enter_context(tc.tile_pool(name="psum_s", bufs=1, space="PSUM"))

        w16 = consts.tile([128, len(plan.mats), CH], F16)
        nc.sync.dma_start(out=w16, in_=w16_in.rearrange("n k m -> k n m"))
        ones_col = consts.tile([128, 1], F16)
        nc.vector.memset(ones_col, 1.0)
        ones_row = consts.tile([1, 128], F16)
        nc.vector.memset(ones_row, 1.0)

        NXG = NCH[0] // GRP  # 8 x-groups per block

        for blk in range(NBLK):
            bb = blk * JB
            # ---- stream x in groups of GRP chunks ----
            x_ts = []
            for g in range(NXG):
                xt = xpool.tile([128, GRP, JB, F], F32, tag="xg")
                for b in range(JB):
                    src = bass.AP(
                        tensor=x_in.tensor,
                        offset=(x_in.offset + (bb + b) * S * F
                                + g * GRP * CH * F),
                        ap=[[F, 128], [CH * F, GRP], [1, F]])
                    nc.sync.dma_start(out=xt[:, :, b], in_=src)
                x16 = xpool.tile([128, GRP, JB, F], F16, tag="xg16")
                nc.vector.tensor_copy(out=x16, in_=xt)
                x_ts.append(x16)

            def x_rhs(ic, r0, r1):
                return x_ts[ic // GRP][r0:r1, ic % GRP]

            # ---- forward ----
            a_tiles = [None]
            d_tiles = []
            for l in range(LEVEL):
                a_nc = NCH[l + 1]
                a_new = apool.tile([128, a_nc, J], F16, tag=f"a{l + 1}")
                d_new = dpool.tile([128, a_nc, J], F16, tag=f"d{l + 1}")
                for grp in _groups(plan.fwd[l]['a']):
                    for part, dst in (('a', a_new), ('d', d_new)):
                        ocs = plan.fwd[l][part]
                        pt = psum.tile([128, GRP, J], F32, tag="pmain")
                        for gi, oc in enumerate(grp):
                            mv, entries = ocs[oc]
                            for i, (ic, r0, r1, mid) in enumerate(entries):
                                lhsT = w16[r0:r1, mid, :mv]
                                rhs = (x_rhs(ic, r0, r1) if l == 0
                                       else a_tiles[l][r0:r1, ic])
                                nc.tensor.matmul(pt[:mv, gi], lhsT, rhs,
                                                 start=(i == 0),
                                                 stop=(i == len(entries) - 1))
                        mv0 = ocs[grp[0]][0]
                        if len(grp) > 1:
                            nc.scalar.copy(out=dst[:, grp[0]:grp[-1] + 1],
                                           in_=pt[:, :len(grp)])
                        else:
                            nc.scalar.copy(out=dst[:mv0, grp[0]],
                                           in_=pt[:mv0, 0])
                a_tiles.append(a_new)
                d_tiles.append(d_new)

            # ---- threshold ----
            d1 = d_tiles[0]
            n1 = LENS[1]
            nf = NCH[1]
            sq = sqpool.tile([128, nf, J], F16, tag="sq")
            nc.vector.tensor_mul(sq, d1, d1)
            ps_sum = psum_s.tile([1, J], F32, tag="pssum")
            ps_sq = psum_s.tile([1, J], F32, tag="pssq")
            for c in range(nf):
                kv = min(CH, n1 - c * CH)
                nc.tensor.matmul(ps_sum, ones_col[:kv, 0:1], d1[:kv, c],
                                 start=(c == 0), stop=(c == nf - 1))
            for c in range(nf):
                kv = min(CH, n1 - c * CH)
                nc.tensor.matmul(ps_sq, ones_col[:kv, 0:1], sq[:kv, c],
                                 start=(c == 0), stop=(c == nf - 1))
            trow = tpool.tile([1, 4, J], F32, tag="trow")
            nc.vector.tensor_scalar_mul(trow[0:1, 0], ps_sum, 1.0 / n1)
            nc.vector.tensor_scalar_mul(trow[0:1, 1], ps_sq, 1.0 / n1)
            nc.vector.tensor_mul(trow[0:1, 2], trow[0:1, 0], trow[0:1, 0])
            nc.vector.tensor_sub(trow[0:1, 3], trow[0:1, 1], trow[0:1, 2])
            nc.vector.tensor_scalar_max(trow[0:1, 3], trow[0:1, 3], 0.0)
            thr_row = tpool.tile([1, J], F16, tag="throw")
            nc.scalar.activation(out=thr_row, in_=trow[0:1, 3],
                                 func=mybir.ActivationFunctionType.Sqrt,
                                 scale=THR_K * THR_K)
            ptb = psum.tile([128, GRP, J], F32, tag="pmain")
            nc.tensor.matmul(ptb[:, 0], ones_row[0:1], thr_row[0:1],
                             start=True, stop=True)
            t_bc = tpool.tile([128, J], F16, tag="tbc")
            nt_bc = tpool.tile([128, J], F16, tag="ntbc")
            nc.scalar.copy(out=t_bc, in_=ptb[:, 0])
            nc.scalar.mul(out=nt_bc, in_=ptb[:, 0], mul=-1.0)

            # ---- soft-threshold in place ----
            def soft_dve(tl, n_elems, nchk):
                rows = min(CH, n_elems)
                tb = _bcast(t_bc, nchk) if nchk > 1 else t_bc[:rows]
                ntb = _bcast(nt_bc, nchk) if nchk > 1 else nt_bc[:rows]
                view = tl if nchk > 1 else tl[:rows, 0]
                u = upool.tile([128, nchk, J], F16, tag="uv")
                uv = u if nchk > 1 else u[:rows, 0]
                nc.vector.tensor_max(uv, view, ntb)
                nc.vector.tensor_tensor(uv, uv, tb, op=mybir.AluOpType.min)
                nc.vector.tensor_sub(view, view, uv)

            import os
            if not os.environ.get("K_NO_SOFT"):
                soft_dve(d1, LENS[1], NCH[1])
                soft_dve(a_tiles[LEVEL], LENS[LEVEL], 1)
                for l in range(2, LEVEL + 1):
                    soft_dve(d_tiles[l - 1], LENS[l], NCH[l])

            # ---- inverse ----
            rec = a_tiles[LEVEL]
            for i in range(LEVEL):
                l = LEVEL - i
                nd, out_len, ocs = plan.inv[i]
                last = (i == LEVEL - 1)
                o_nc = (out_len + CH - 1) // CH
                if not last:
                    out_t = rpool.tile([128, o_nc, J], F16, tag=f"rec{l}")
                d_t = d_tiles[l - 1]
                for grp in _groups(ocs):
                    pt = psum.tile([128, GRP, J], F32, tag="pmain")
                    for gi, oc in enumerate(grp):
                        mv, entries = ocs[oc]
                        for k, (srct, ic, r0, r1, mid) in enumerate(entries):
                            src = rec if srct == 'a' else d_t
                            nc.tensor.matmul(pt[:mv, gi], w16[r0:r1, mid, :mv],
                                             src[r0:r1, ic],
                                             start=(k == 0),
                                             stop=(k == len(entries) - 1))
                    if last:
                        ot = opool.tile([128, GRP, JB, F], F32, tag="outg")
                        nc.scalar.copy(
                            out=ot.rearrange("p g b f -> p g (b f)"),
                            in_=pt[:, :len(grp)])
                        for b in range(JB):
                            dst = bass.AP(
                                tensor=y_out.tensor,
                                offset=(y_out.offset + (bb + b) * S * F
                                        + grp[0] * CH * F),
                                ap=[[F, 128], [CH * F, len(grp)], [1, F]])
                            nc.sync.dma_start(out=dst,
                                              in_=ot[:, :len(grp), b])
                    else:
                        mv0 = ocs[grp[0]][0]
                        if len(grp) > 1:
                            nc.scalar.copy(out=out_t[:, grp[0]:grp[-1] + 1],
                                           in_=pt[:, :len(grp)])
                        else:
                            nc.scalar.copy(out=out_t[:mv0, grp[0]],
                                           in_=pt[:mv0, 0])
                if not last:
                    rec = out_t


def _build_kernel():
    plan = PLAN
    nc = bacc.Bacc("TRN2", target_bir_lowering=False, debug=False,
                   num_devices=NCORES)
    x_in = nc.dram_tensor("x", [BPC, S, F], F32, kind="ExternalInput").ap()
    y_out = nc.dram_tensor("out", [BPC, S, F], F32, kind="ExternalOutput").ap()

    w16_np = np.zeros((len(plan.mats), 128, CH), dtype=np.float16)
    for i, (m, r0) in enumerate(plan.mats):
        w16_np[i, r0:r0 + m.shape[0], :m.shape[1]] = m.astype(np.float16)
    w16_in = nc.dram_tensor("w16", list(w16_np.shape), F16,
                            kind="ExternalInput").ap()

    with tile.TileContext(nc) as tc:
        _kernel_body(tc, y_out, x_in, w16_in, plan)
    nc.compile()
    return nc, w16_np


_CACHE = {}


def kernel(x, trace=False, tmpdir=None):
    x = np.ascontiguousarray(x, dtype=np.float32)
    assert x.shape == (B, S, F)
    if "nc" not in _CACHE:
        _CACHE["nc"], _CACHE["w16"] = _build_kernel()
    nc = _CACHE["nc"]
    in_maps = [{"x": x[c * BPC:(c + 1) * BPC], "w16": _CACHE["w16"]}
               for c in range(NCORES)]
    res = run_bass_kernel_spmd(nc, in_maps, list(range(NCORES)),
                               trace=trace, tmpdir=tmpdir)
    out = np.concatenate([res.results[c]["out"] for c in range(NCORES)],
                         axis=0)
    if trace:
        _CACHE["last_results"] = res
    return out
